# revision 1
# baseline (speedup 1.0000x reference)
"""Trainium2 Bass kernel for GQA attention with ALiBi + sliding window + QK-RMSNorm.

Sharding: tensor-parallel over heads across 8 cores. Core c owns q-heads
[4c,4c+4) and kv-head c. Each core computes a partial output through its
wo column-shard; host sums the 8 partials. The RMSNorm over the full
(flattened-heads) axis needs a cross-core sum-of-squares -> tiny on-device
AllReduce (2x4096 f32).

All matmuls run as float32r (FP22, full PE rate at free-dim>=256).
ALiBi bias + causal/window mask are folded into the score PSUM via an
identity-matmul add of a host-precomputed bias tensor (masked = -1e30).
"""
import sys, os
sys.path.insert(0, "/opt/trn_rl_repo")

import numpy as np

B, T, DIM = 2, 2048, 2048
NH, NKV, HD = 32, 8, 64
WINDOW = 1024
EPS = 1e-6
T4 = B * T            # 4096 flattened tokens
QH = NH // 8          # 4 q heads per core
QD = QH * HD          # 256 q dims per core
TP = 256              # projection token tile
TQ = 256              # attention query tile
NKT = DIM // 128      # 16 k-tiles for projections
BIAS_W = 1408         # bias cols: u = tt + (t0-s0) + 128

_CACHE = {}


def _build_bass():
    from concourse import bass, bacc, mybir
    from concourse.tile import TileContext

    dt = mybir.dt.float32
    dtr = mybir.dt.float32r
    AF = mybir.ActivationFunctionType

    nc = bacc.Bacc("TRN2", target_bir_lowering=False, debug=False,
                   num_devices=8)

    xT = nc.dram_tensor("xT", [DIM, T4], dtr, kind="ExternalInput")
    wT = nc.dram_tensor("wT", [DIM, QD + 2 * HD], dtr, kind="ExternalInput")
    woT = nc.dram_tensor("woT", [QD, DIM], dtr, kind="ExternalInput")
    qnw = nc.dram_tensor("qnw", [1, QD], dtr, kind="ExternalInput")
    knw = nc.dram_tensor("knw", [1, 128], dtr, kind="ExternalInput")
    biasT = nc.dram_tensor("biasT", [QH, 128, BIAS_W], dtr, kind="ExternalInput")
    ident_in = nc.dram_tensor("ident", [128, 128], dtr, kind="ExternalInput")
    ones_in = nc.dram_tensor("ones2", [2, 128], dtr, kind="ExternalInput")
    onesc_in = nc.dram_tensor("ones_col", [128, 1], dtr, kind="ExternalInput")
    sc_in = nc.dram_tensor("sc_col", [128, 64], dt, kind="ExternalInput")
    bi_in = nc.dram_tensor("bi_col", [128, 64], dt, kind="ExternalInput")
    out_d = nc.dram_tensor("out", [T4, DIM], dt, kind="ExternalOutput")

    with TileContext(nc) as tc:
        with (
            tc.tile_pool(name="consts", bufs=1) as cp,
            tc.tile_pool(name="persist", bufs=1) as pp,
            tc.tile_pool(name="xin", bufs=2) as xp,
            tc.tile_pool(name="work", bufs=2) as wk,
            tc.tile_pool(name="expp", bufs=6) as ep,
            tc.tile_pool(name="outp", bufs=3) as op_,
            tc.tile_pool(name="dram", bufs=1, space="DRAM") as dp,
        ):
            # ---- constants / weights, loaded once ----
            wtiles = []
            for kt in range(NKT):
                t = cp.tile([128, QD + 2 * HD], dtr, tag=f"w{kt}")
                nc.sync.dma_start(t[:], wT[kt * 128:(kt + 1) * 128, :])
                wtiles.append(t)
            wo_sb = []
            for p in range(2):
                t = cp.tile([128, DIM], dtr, tag=f"wo{p}")
                nc.sync.dma_start(t[:], woT[p * 128:(p + 1) * 128, :])
                wo_sb.append(t)
            bias_sb = []
            for h in range(QH):
                t = cp.tile([128, BIAS_W], dtr, tag=f"b{h}")
                nc.sync.dma_start(t[:], biasT[h])
                bias_sb.append(t)
            ident = cp.tile([128, 128], dtr, tag="id")
            nc.sync.dma_start(ident[:], ident_in[:])
            ones2 = cp.tile([2, 128], dtr, tag="on")
            nc.sync.dma_start(ones2[:], ones_in[:])
            qnw_sb = cp.tile([1, QD], dtr, tag="qnw")
            nc.sync.dma_start(qnw_sb[:], qnw[:])
            knw_sb = cp.tile([1, 128], dtr, tag="knw")
            nc.sync.dma_start(knw_sb[:], knw[:])
            sc_col = cp.tile([128, 64], dt, tag="sc")
            nc.sync.dma_start(sc_col[:], sc_in[:])
            bi_col = cp.tile([128, 64], dt, tag="bi")
            nc.sync.dma_start(bi_col[:], bi_in[:])
            ones_col = cp.tile([128, 1], dtr, tag="oc")
            nc.sync.dma_start(ones_col[:], onesc_in[:])

            # ---- persistent activations ----
            # q heads packed 2-per-tile: head h -> tile h//2, rows 64*(h%2)
            qts2 = [pp.tile([128, T4], dtr, tag=f"q{p}", name=f"q{p}")
                    for p in range(2)]
            kT2 = pp.tile([128, T4], dtr, tag="kT")
            vaug = []
            for sb in range(T4 // 128):
                t = pp.tile([128, HD + 1], dtr, tag=f"v{sb}")
                nc.sync.dma_start(t[:, HD:HD + 1], onesc_in[:])
                vaug.append(t)

            cc_in = dp.tile([2, T4], dt)
            cc_out = dp.tile([2, T4], dt)
            rs_dram = dp.tile([2, T4], dtr)

            # ================= phase 1: projections + sumsq =================
            with (
                tc.tile_pool(name="ps_proj", bufs=3, space="PSUM") as pj,
                tc.tile_pool(name="ps_ss", bufs=1, space="PSUM") as pss,
                tc.tile_pool(name="ps_tr", bufs=2, space="PSUM") as ptr,
            ):
                for it in range(T4 // TP):
                    ts0 = it * TP
                    tsl = slice(ts0, ts0 + TP)
                    xts = []
                    for kt in range(NKT):
                        t = xp.tile([128, TP], dtr, tag=f"x{kt}")
                        nc.sync.dma_start(
                            t[:], xT[kt * 128:(kt + 1) * 128, tsl])
                        xts.append(t)
                    sspsum = pss.tile([1, TP], dt, tag="ss")
                    sskp = pss.tile([1, TP], dt, tag="ssk_ps")
                    vtmp = None
                    for mt in range(3):
                        ppsum = pj.tile([128, TP], dt, tag="pj")
                        for kt in range(NKT):
                            nc.tensor.matmul(
                                ppsum[:],
                                wtiles[kt][:, mt * 128:(mt + 1) * 128]
                                ,
                                xts[kt][:],
                                start=(kt == 0), stop=(kt == NKT - 1))
                        if mt < 2:
                            nc.any.tensor_copy(qts2[mt][:, tsl], ppsum[:])
                            sq = wk.tile([128, TP], dtr, tag="sq")
                            nc.vector.tensor_mul(
                                sq[:], qts2[mt][:, tsl], qts2[mt][:, tsl])
                            nc.tensor.matmul(
                                sspsum[0:1, :], ones_col[:],
                                sq[:],
                                start=(mt == 0), stop=(mt == 1))
                        else:
                            nc.any.tensor_copy(kT2[0:64, tsl], ppsum[0:64, :])
                            nc.any.tensor_copy(kT2[64:128, tsl],
                                               ppsum[0:64, :])
                            sqk = wk.tile([64, TP], dtr, tag="sqk")
                            nc.vector.tensor_mul(
                                sqk[:], kT2[0:64, tsl], kT2[0:64, tsl])
                            nc.tensor.matmul(
                                sskp[:],
                                ones_col[0:64, :],
                                sqk[:], start=True, stop=True)
                            vtmp = wk.tile([64, TP], dt, tag="vt")
                            nc.any.tensor_copy(vtmp[:], ppsum[64:128, :])
                    # sumsq partials to DRAM for the AllReduce (via SBUF)
                    ssq = wk.tile([1, TP], dt, tag="ssq")
                    nc.vector.tensor_copy(ssq[:], sspsum[0:1, :])
                    ssk = wk.tile([1, TP], dt, tag="ssk")
                    nc.vector.tensor_copy(ssk[:], sskp[:])
                    nc.sync.dma_start(cc_in[0:1, tsl], ssq[:])
                    nc.sync.dma_start(cc_in[1:2, tsl], ssk[:])
                    # transpose V into [s, d] layout (+ ones column pre-set)
                    for j in range(TP // 128):
                        tp_ = ptr.tile([128, 64], dt, tag="tr")
                        nc.tensor.transpose(
                            tp_[:], vtmp[:, j * 128:(j + 1) * 128],
                            ident[0:64, 0:64].bitcast(dt))
                        nc.any.tensor_copy(
                            vaug[(ts0 + j * 128) // 128][:, 0:HD], tp_[:])

            # ================= phase 2: AllReduce + rsqrt =================
            nc.gpsimd.collective_compute(
                "AllReduce", mybir.AluOpType.add,
                replica_groups=[list(range(8))],
                ins=[cc_in.opt()], outs=[cc_out.opt()])

            # rectangular [128,64] layout: rows 0:64 = q-ss (64 tokens per
            # partition), rows 64:128 = k-ss.
            ss_rect = pp.tile([128, 64], dt, tag="ssr")
            nc.sync.dma_start(
                ss_rect[:], cc_out[:].rearrange("r (p c) -> (r p) c", c=64))
            # rsq = rsqrt(ss*sc + bi); q rows: sc=1/32 (folds the 1/8 score
            # scale), bi=64*eps; k rows: sc=1/512, bi=eps. Newton-refined.
            vaff = pp.tile([128, 64], dt, tag="vaff")
            nc.vector.tensor_mul(vaff[:], ss_rect[:], sc_col[:])
            nc.vector.tensor_add(vaff[:], vaff[:], bi_col[:])
            s1 = pp.tile([128, 64], dt, tag="s1")
            nc.scalar.activation(s1[:], vaff[:], AF.Sqrt)
            y0 = pp.tile([128, 64], dt, tag="y0")
            nc.vector.reciprocal(y0[:], s1[:])
            t1 = pp.tile([128, 64], dt, tag="t1")
            nc.vector.tensor_mul(t1[:], y0[:], y0[:])
            nc.vector.tensor_mul(t1[:], t1[:], vaff[:])
            nc.scalar.activation(t1[:], t1[:], AF.Copy, bias=1.5, scale=-0.5)
            rs_fin = pp.tile([128, 64], dtr, tag="rsf")
            nc.vector.tensor_mul(rs_fin[:], y0[:], t1[:])
            nc.sync.dma_start(
                rs_dram[:].rearrange("r (p c) -> (r p) c", c=64), rs_fin[:])

            # ============ phase 3: normalize + attention + wo ============
            with (
                tc.tile_pool(name="ps_sc", bufs=3, space="PSUM") as psc,
                tc.tile_pool(name="ps_o", bufs=2, space="PSUM") as po,
                tc.tile_pool(name="ps_wo", bufs=3, space="PSUM") as pw,
            ):
                # normalize q,k in place: q *= qnw (x) rs_q  (rank-1 PE tile)
                for i in range(T4 // TQ):
                    tsl = slice(i * TQ, (i + 1) * TQ)
                    rsq_t = wk.tile([1, TQ], dtr, tag="rsq")
                    nc.sync.dma_start(rsq_t[:], rs_dram[0:1, tsl])
                    rsk_t = wk.tile([1, TQ], dtr, tag="rsk")
                    nc.sync.dma_start(rsk_t[:], rs_dram[1:2, tsl])
                    for p in range(2):
                        scp = psc.tile([128, TQ], dt, tag="sc")
                        nc.tensor.matmul(
                            scp[:],
                            qnw_sb[0:1, p * 128:(p + 1) * 128],
                            rsq_t[:], start=True, stop=True)
                        nc.vector.tensor_mul(qts2[p][:, tsl], qts2[p][:, tsl],
                                             scp[:])
                    sck = psc.tile([128, TQ], dt, tag="sc")
                    nc.tensor.matmul(sck[:], knw_sb[0:1, :],
                                     rsk_t[:],
                                     start=True, stop=True)
                    nc.vector.tensor_mul(kT2[:, tsl], kT2[:, tsl], sck[:])

                # attention
                for b in range(B):
                    for i in range(T // TQ):
                        t0 = i * TQ
                        g0 = b * T + t0
                        s0lo = max(0, t0 - WINDOW)
                        nblk = (t0 - s0lo) // 128 + 2
                        opair = [op_.tile([128, TQ], dtr, tag=f"op{p}",
                                          name=f"op{p}")
                                 for p in range(2)]
                        for h in range(QH):
                            qrow = (h % 2) * 64
                            opsum = po.tile([128, TQ], dt, tag="o")
                            for jp in range(0, nblk, 2):
                                scp = psc.tile([128, 2 * TQ], dt, tag="sc")
                                for dj in range(2):
                                    j = jp + dj
                                    s0 = s0lo + j * 128
                                    gs = b * T + s0
                                    col = slice(dj * TQ, (dj + 1) * TQ)
                                    nc.tensor.matmul(
                                        scp[:, col],
                                        kT2[qrow:qrow + 64, gs:gs + 128],
                                        qts2[h // 2][qrow:qrow + 64,
                                                     g0:g0 + TQ],
                                        start=True, stop=False)
                                    u0 = t0 - s0 + 128
                                    nc.tensor.matmul(
                                        scp[:, col], ident[:],
                                        bias_sb[h][:, u0:u0 + TQ],
                                        start=False, stop=True)
                                et = ep.tile([128, 2 * TQ], dtr, tag="e")
                                nc.scalar.activation(et[:], scp[:], AF.Exp)
                                for dj in range(2):
                                    j = jp + dj
                                    gs = b * T + s0lo + j * 128
                                    nc.tensor.matmul(
                                        opsum[0:65, :],
                                        vaug[gs // 128][:],
                                        et[:, dj * TQ:(dj + 1) * TQ],
                                        start=(j == 0), stop=(j == nblk - 1))
                            zf = wk.tile([1, TQ], dt, tag="zf")
                            nc.vector.reciprocal(zf[:], opsum[64:65, :])
                            zinv = wk.tile([1, TQ], dtr, tag="z")
                            nc.vector.tensor_copy(zinv[:], zf[:])
                            zbc = psc.tile([64, TQ], dt, tag="sc")
                            nc.tensor.matmul(
                                zbc[:], ones2[0:1, 0:64],
                                zinv[:], start=True, stop=True)
                            prow = (h % 2) * 64
                            pair = h // 2
                            nc.any.tensor_copy(
                                opair[pair][prow:prow + 64, :], opsum[0:64, :])
                            nc.vector.tensor_mul(
                                opair[pair][prow:prow + 64, :],
                                opair[pair][prow:prow + 64, :], zbc[:])
                        # wo projection for these 256 tokens
                        for m in range(TQ // 128):
                            for e in range(DIM // 512):
                                wop = pw.tile([128, 512], dt, tag="wo")
                                for p in range(2):
                                    nc.tensor.matmul(
                                        wop[:],
                                        opair[p][:, m * 128:(m + 1) * 128]
                                        ,
                                        wo_sb[p][:, e * 512:(e + 1) * 512]
                                        ,
                                        start=(p == 0), stop=(p == 1))
                                ost = op_.tile([128, 512], dt, tag="os")
                                nc.any.tensor_copy(ost[:], wop[:])
                                nc.sync.dma_start(
                                    out_d[g0 + m * 128:g0 + (m + 1) * 128,
                                          e * 512:(e + 1) * 512], ost[:])
    nc.finalize()
    return nc


def _host_inputs(x, wq, wk, wv, wo, q_norm_w, k_norm_w):
    f32 = np.float32
    x = np.asarray(x, f32)
    xT = np.ascontiguousarray(x.reshape(T4, DIM).T)
    r = 2.0 ** (-8.0 / NH)
    slopes = np.asarray([r ** i for i in range(NH)], f32)
    ident = np.eye(128, dtype=f32)
    ones2 = np.ones((2, 128), f32)
    sc_col = np.concatenate([np.full((64, 64), 1.0 / 32.0, f32),
                             np.full((64, 64), 1.0 / 512.0, f32)])
    bi_col = np.concatenate([np.full((64, 64), 64.0 * EPS, f32),
                             np.full((64, 64), EPS, f32)])
    ds = np.arange(128, dtype=np.int64)[:, None]
    ui = np.arange(BIAS_W, dtype=np.int64)[None, :] - 128
    dist = ui - ds  # = t - s
    allowed = (dist >= 0) & (dist <= WINDOW)
    in_maps = []
    for c in range(8):
        wTc = np.ascontiguousarray(np.concatenate([
            np.asarray(wq, f32)[c * QD:(c + 1) * QD],
            np.asarray(wk, f32)[c * HD:(c + 1) * HD],
            np.asarray(wv, f32)[c * HD:(c + 1) * HD]], 0).T)
        woTc = np.ascontiguousarray(
            np.asarray(wo, f32)[:, c * QD:(c + 1) * QD].T)
        bias = np.empty((QH, 128, BIAS_W), f32)
        for h in range(QH):
            sl = slopes[4 * c + h]
            bias[h] = np.where(allowed, (-sl * dist).astype(f32), f32(-1e30))
        in_maps.append({
            "xT": xT,
            "wT": wTc,
            "woT": woTc,
            "qnw": np.asarray(q_norm_w, f32)[c * QD:(c + 1) * QD]
            .reshape(1, QD),
            "knw": np.tile(np.asarray(k_norm_w, f32)[c * HD:(c + 1) * HD],
                           2).reshape(1, 128),
            "biasT": bias,
            "ident": ident,
            "ones2": ones2,
            "ones_col": np.ones((128, 1), f32),
            "sc_col": sc_col,
            "bi_col": bi_col,
        })
    return in_maps


def kernel(x, wq, wk, wv, wo, q_norm_w, k_norm_w):
    from concourse.bass_utils import run_bass_kernel_spmd
    if "nc" not in _CACHE:
        _CACHE["nc"] = _build_bass()
    nc = _CACHE["nc"]
    in_maps = _host_inputs(x, wq, wk, wv, wo, q_norm_w, k_norm_w)
    res = run_bass_kernel_spmd(nc, in_maps, core_ids=list(range(8)))
    out = np.zeros((T4, DIM), np.float64)
    for c in range(8):
        out += res.results[c]["out"].astype(np.float64)
    return out.reshape(B, T, DIM).astype(np.float32)



# revision 21
# speedup vs baseline: 1.9605x; 1.9605x over previous
"""Trainium2 Bass kernel for GQA attention with ALiBi + sliding window + QK-RMSNorm.

Sharding: tensor-parallel over heads across 8 cores. Core c owns q-heads
[4c,4c+4) and kv-head c. Each core computes a partial output through its
wo column-shard; host sums the 8 partials. The RMSNorm over the full
(flattened-heads) axis needs a cross-core sum-of-squares -> tiny on-device
AllReduce (2x4096 f32).

v2 design:
- Projections/wo run weight-stationary in bf16 (few LDWEIGHTS, N=512 moving).
- ALiBi bias slope*(s-t) is folded INTO the score matmul via two augmented
  contraction rows: k_hat=[k; s; 1], q_hat=[q*rsq; slope; -slope*t]. Only
  the 2 diagonal + up-to-2 window-edge key blocks per query tile need a
  mask, applied as a DVE add of a constant -1e30 tile onto the score PSUM.
- Scores: k_hat block stationary, shared by the core's 4 q-heads (fp32r,
  N=256). exp on scalar engine -> bf16 P. PV: v_hat [128,65] stationary
  (ones column = softmax denominator), bf16. Softmax divide via
  reciprocal_approx_fast + rank-1 broadcast.
"""
import sys, os
sys.path.insert(0, "/opt/trn_rl_repo")

import numpy as np

B, T, DIM = 2, 2048, 2048
NH, NKV, HD = 32, 8, 64
WINDOW = 1024
EPS = 1e-6
T4 = B * T            # 4096 flattened tokens
QH = NH // 8          # 4 q heads per core
QD = QH * HD          # 256 q dims per core
TC = 512              # projection token chunk
NCH = T4 // TC        # 8 chunks
TQ = 256              # attention query tile (pair of 128-blocks)
NKT = DIM // 128      # 16 k-tiles for projections

_CACHE = {}


def _build_bass():
    DBG = bool(os.environ.get("KDEBUG"))
    from concourse import bass, bacc, mybir
    from concourse.tile import TileContext

    dt = mybir.dt.float32
    dtr = mybir.dt.float32r
    bf = mybir.dt.bfloat16
    AF = mybir.ActivationFunctionType

    nc = bacc.Bacc("TRN2", target_bir_lowering=False, debug=False,
                   num_devices=8)

    xT = nc.dram_tensor("xT", [DIM, T4], bf, kind="ExternalInput")
    wT = nc.dram_tensor("wT", [DIM, QD + 2 * HD], bf, kind="ExternalInput")
    woT = nc.dram_tensor("woT", [QD, DIM], bf, kind="ExternalInput")
    qnw = nc.dram_tensor("qnw", [1, QD], dtr, kind="ExternalInput")
    knw = nc.dram_tensor("knw", [1, HD], dtr, kind="ExternalInput")
    aug = nc.dram_tensor("aug", [10, T4], dtr, kind="ExternalInput")
    masks = nc.dram_tensor("masks", [4, 128, 2 * TQ], dt, kind="ExternalInput")
    onesc_in = nc.dram_tensor("ones_col", [128, 1], dtr, kind="ExternalInput")
    onesr_in = nc.dram_tensor("ones_row", [1, 64], dtr, kind="ExternalInput")
    ident_in = nc.dram_tensor("ident", [64, 64], dt, kind="ExternalInput")
    sc_in = nc.dram_tensor("sc_col", [128, 64], dt, kind="ExternalInput")
    bi_in = nc.dram_tensor("bi_col", [128, 64], dt, kind="ExternalInput")
    out_d = nc.dram_tensor("out", [DIM, T4], bf, kind="ExternalOutput")
    if DBG:
        dbg_d = nc.dram_tensor("dbg", [10, 128, 2 * TQ], dt,
                               kind="ExternalOutput")

    with TileContext(nc) as tc:
        with (
            tc.tile_pool(name="consts", bufs=1) as cp,
            tc.tile_pool(name="persist", bufs=1) as pp,
            tc.tile_pool(name="xin", bufs=1 if DBG else 2) as xp,
            tc.tile_pool(name="work", bufs=1 if DBG else 2) as wk,
            tc.tile_pool(name="expp", bufs=3 if DBG else 4) as ep,
            tc.tile_pool(name="outp", bufs=3) as op_,
            tc.tile_pool(name="dram", bufs=1, space="DRAM") as dp,
        ):
            # ---- constants / weights, loaded once ----
            wtiles = []
            for kt in range(NKT):
                t = cp.tile([128, QD + 2 * HD], bf, tag=f"w{kt}")
                nc.sync.dma_start(t[:], wT[kt * 128:(kt + 1) * 128, :])
                wtiles.append(t)
            wo_sb = []
            for p in range(2):
                t = cp.tile([128, DIM], bf, tag=f"wo{p}")
                nc.sync.dma_start(t[:], woT[p * 128:(p + 1) * 128, :])
                wo_sb.append(t)
            mask_sb = []
            for m in range(4):
                t = cp.tile([128, 2 * TQ], dt, tag=f"m{m}")
                nc.sync.dma_start(t[:], masks[m])
                mask_sb.append(t)
            qnw_sb = cp.tile([1, QD], dtr, tag="qnw")
            nc.sync.dma_start(qnw_sb[:], qnw[:])
            knw_sb = cp.tile([1, HD], dtr, tag="knw")
            nc.sync.dma_start(knw_sb[:], knw[:])
            sc_col = cp.tile([128, 64], dt, tag="sc")
            nc.sync.dma_start(sc_col[:], sc_in[:])
            bi_col = cp.tile([128, 64], dt, tag="bi")
            nc.sync.dma_start(bi_col[:], bi_in[:])
            ones_col = cp.tile([128, 1], dtr, tag="oc")
            nc.sync.dma_start(ones_col[:], onesc_in[:])
            ones_row = cp.tile([1, 64], dtr, tag="or")
            nc.sync.dma_start(ones_row[:], onesr_in[:])
            ident = cp.tile([64, 64], dt, tag="id")
            nc.sync.dma_start(ident[:], ident_in[:])

            # ---- persistent activations ----
            # q_hat per head: rows 0:64 = q dims, row 64 = slope_h,
            # row 65 = -slope_h * t
            qh_t = []
            for h in range(QH):
                t = pp.tile([66, T4], dtr, tag=f"q{h}", name=f"q{h}")
                nc.sync.dma_start(t[64:65, :], aug[2 + h:3 + h, :])
                nc.sync.dma_start(t[65:66, :], aug[6 + h:7 + h, :])
                qh_t.append(t)
            # k_hat: rows 0:64 = k dims, row 64 = s, row 65 = 1
            kh = pp.tile([66, T4], dtr, tag="kT", name="kT")
            nc.sync.dma_start(kh[64:65, :], aug[0:1, :])
            nc.sync.dma_start(kh[65:66, :], aug[1:2, :])
            # v_hat per 128-token key block: [128, 64+1] bf16, col 64 = 1
            vaug = []
            for sb in range(T4 // 128):
                t = pp.tile([128, HD + 1], bf, tag=f"v{sb}")
                nc.vector.memset(t[:, HD:HD + 1], 1.0)
                vaug.append(t)
            # attention output (divided), per head-pair: rows 0:64 head 2p,
            # rows 64:128 head 2p+1
            o2 = [pp.tile([128, T4], bf, tag=f"o{p}", name=f"o{p}")
                  for p in range(2)]

            cc_in = dp.tile([2, T4], dt)
            cc_out = dp.tile([2, T4], dt)
            rs_dram = dp.tile([2, T4], dtr)

            # ================= phase 1: projections + sumsq =================
            with (
                tc.tile_pool(name="ps_proj", bufs=2, space="PSUM") as pj,
                tc.tile_pool(name="ps_ss", bufs=2, space="PSUM") as pss,
                tc.tile_pool(name="ps_tr", bufs=2, space="PSUM") as ptr,
            ):
                for ch in range(NCH):
                    ts0 = ch * TC
                    tsl = slice(ts0, ts0 + TC)
                    xts = []
                    for kt in range(NKT):
                        t = xp.tile([128, TC], bf, tag=f"x{kt}")
                        nc.sync.dma_start(
                            t[:], xT[kt * 128:(kt + 1) * 128, tsl])
                        xts.append(t)
                    sspsum = pss.tile([1, TC], dt, tag="ss")
                    sskp = pss.tile([1, TC], dt, tag="ssk_ps")
                    for mt in range(3):
                        ppsum = pj.tile([128, TC], dt, tag="pj")
                        for kt in range(NKT):
                            nc.tensor.matmul(
                                ppsum[:],
                                wtiles[kt][:, mt * 128:(mt + 1) * 128],
                                xts[kt][:],
                                start=(kt == 0), stop=(kt == NKT - 1))
                        if mt < 2:
                            for hh in range(2):
                                h = 2 * mt + hh
                                nc.any.tensor_copy(
                                    qh_t[h][0:64, tsl],
                                    ppsum[hh * 64:(hh + 1) * 64, :])
                                sq = wk.tile([64, TC], dtr, tag="sq",
                                             name="sq")
                                nc.vector.tensor_mul(
                                    sq[:], qh_t[h][0:64, tsl],
                                    qh_t[h][0:64, tsl])
                                nc.tensor.matmul(
                                    sspsum[:], ones_col[0:64, :], sq[:],
                                    start=(h == 0), stop=(h == 3))
                        else:
                            nc.any.tensor_copy(kh[0:64, tsl], ppsum[0:64, :])
                            sqk = wk.tile([64, TC], dtr, tag="sqk")
                            nc.vector.tensor_mul(
                                sqk[:], kh[0:64, tsl], kh[0:64, tsl])
                            nc.tensor.matmul(
                                sskp[:], ones_col[0:64, :], sqk[:],
                                start=True, stop=True)
                            vtmp = wk.tile([64, TC], dt, tag="vt")
                            nc.any.tensor_copy(vtmp[:], ppsum[64:128, :])
                            # transpose V into [s, d] layout
                            for j in range(TC // 128):
                                tp_ = ptr.tile([128, 64], dt, tag="tr")
                                nc.tensor.transpose(
                                    tp_[:], vtmp[:, j * 128:(j + 1) * 128],
                                    ident[:])
                                nc.any.tensor_copy(
                                    vaug[(ts0 + j * 128) // 128][:, 0:HD],
                                    tp_[:])
                    ssq = wk.tile([1, TC], dt, tag="ssq")
                    nc.vector.tensor_copy(ssq[:], sspsum[:])
                    ssk = wk.tile([1, TC], dt, tag="ssk")
                    nc.vector.tensor_copy(ssk[:], sskp[:])
                    nc.sync.dma_start(cc_in[0:1, tsl], ssq[:])
                    nc.sync.dma_start(cc_in[1:2, tsl], ssk[:])

            # ================= phase 2: AllReduce + rsqrt =================
            nc.gpsimd.collective_compute(
                "AllReduce", mybir.AluOpType.add,
                replica_groups=[list(range(8))],
                ins=[cc_in.opt()], outs=[cc_out.opt()])

            # rectangular [128,64]: rows 0:64 q-ss, rows 64:128 k-ss;
            # partition p holds tokens 64p..64p+63 of its half.
            ss_rect = pp.tile([128, 64], dt, tag="ssr")
            nc.sync.dma_start(
                ss_rect[:], cc_out[:].rearrange("r (p c) -> (r p) c", c=64))
            # rsq = rsqrt(ss*sc + bi); q rows: sc=1/32 (folds the 1/8 score
            # scale), bi=64*eps; k rows: sc=1/512, bi=eps. Newton-refined.
            vaff = pp.tile([128, 64], dt, tag="vaff")
            nc.vector.tensor_mul(vaff[:], ss_rect[:], sc_col[:])
            nc.vector.tensor_add(vaff[:], vaff[:], bi_col[:])
            s1 = pp.tile([128, 64], dt, tag="s1")
            nc.scalar.activation(s1[:], vaff[:], AF.Sqrt)
            y0 = pp.tile([128, 64], dt, tag="y0")
            nc.vector.reciprocal(y0[:], s1[:])
            t1 = pp.tile([128, 64], dt, tag="t1")
            nc.vector.tensor_mul(t1[:], y0[:], y0[:])
            nc.vector.tensor_mul(t1[:], t1[:], vaff[:])
            nc.scalar.activation(t1[:], t1[:], AF.Copy, bias=1.5, scale=-0.5)
            rs_fin = pp.tile([128, 64], dtr, tag="rsf")
            nc.vector.tensor_mul(rs_fin[:], y0[:], t1[:])
            nc.sync.dma_start(
                rs_dram[:].rearrange("r (p c) -> (r p) c", c=64), rs_fin[:])

            # ============ phase 3: normalize q,k in place ============
            with tc.tile_pool(name="ps_nm", bufs=3, space="PSUM") as pn:
                for ch in range(NCH):
                    tsl = slice(ch * TC, (ch + 1) * TC)
                    rsq_t = wk.tile([1, TC], dtr, tag="rsq")
                    nc.sync.dma_start(rsq_t[:], rs_dram[0:1, tsl])
                    rsk_t = wk.tile([1, TC], dtr, tag="rsk")
                    nc.sync.dma_start(rsk_t[:], rs_dram[1:2, tsl])
                    for h in range(QH):
                        zq = pn.tile([64, TC], dt, tag="z")
                        nc.tensor.matmul(
                            zq[:], qnw_sb[0:1, h * 64:(h + 1) * 64],
                            rsq_t[:], start=True, stop=True)
                        nc.vector.tensor_mul(
                            qh_t[h][0:64, tsl], qh_t[h][0:64, tsl], zq[:])
                    zk = pn.tile([64, TC], dt, tag="z")
                    nc.tensor.matmul(zk[:], knw_sb[:], rsk_t[:],
                                     start=True, stop=True)
                    nc.vector.tensor_mul(kh[0:64, tsl], kh[0:64, tsl], zk[:])

            # ================= phase 4: attention =================
            with (
                tc.tile_pool(name="ps_sc", bufs=3, space="PSUM") as psc,
                tc.tile_pool(name="ps_o", bufs=2, space="PSUM") as po,
                tc.tile_pool(name="ps_z", bufs=1, space="PSUM") as pz,
            ):
                for b in range(B):
                    for i in range(T // TQ):
                        t0 = i * TQ
                        g0 = b * T + t0
                        s0lo = max(0, t0 - WINDOW)
                        nblk = (t0 - s0lo) // 128 + 2
                        opair = [po.tile([65, 2 * TQ], dt, tag=f"op{p}",
                                          name=f"op{p}")
                                 for p in range(2)]
                        pv_pend = []
                        for j in range(nblk):
                            gs = b * T + s0lo + j * 128
                            scp = [psc.tile([128, 2 * TQ], dt, tag="sc",
                                            name="sc")
                                   for _ in range(2)]
                            for h in range(QH):
                                nc.tensor.matmul(
                                    scp[h // 2][:, (h % 2) * TQ:
                                                (h % 2 + 1) * TQ],
                                    kh[0:66, gs:gs + 128],
                                    qh_t[h][0:66, g0:g0 + TQ],
                                    start=True, stop=True)
                            if j == nblk - 2:
                                m = mask_sb[1]
                            elif j == nblk - 1:
                                m = mask_sb[2]
                            elif j == 0 and t0 >= WINDOW:
                                m = mask_sb[0]
                            elif j == 1 and t0 >= WINDOW:
                                m = mask_sb[3]
                            else:
                                m = None
                            pt = [ep.tile([128, 2 * TQ], bf, tag="e", name="e")
                                  for _ in range(2)]
                            for p in range(2):
                                if m is not None:
                                    nc.vector.tensor_add(
                                        scp[p][:], scp[p][:], m[:])
                                nc.scalar.activation(pt[p][:], scp[p][:],
                                                     AF.Exp)
                            # software-pipeline: PV of previous block runs
                            # while this block's exp is on the scalar engine
                            if pv_pend:
                                pgs, ppt = pv_pend.pop()
                                for p_ in range(2):
                                    nc.tensor.matmul(
                                        opair[p_][0:65, :],
                                        vaug[pgs // 128][:], ppt[p_][:],
                                        start=(pgs == b * T + s0lo),
                                        stop=False)
                            if DBG and b == 0 and i == 0:
                                for p_ in range(2):
                                    dtmp = wk.tile([128, 2 * TQ], dt,
                                                   tag="dbg", name="dbg",
                                                   bufs=4)
                                    nc.vector.tensor_copy(
                                        dtmp[:], scp[p_][:])
                                    nc.sync.dma_start(
                                        dbg_d[2 * j + p_], dtmp[:])
                                dt2 = wk.tile([128, 2 * TQ], dt, tag="dbg2",
                                              name="dbg2", bufs=2)
                                nc.vector.tensor_copy(dt2[:], pt[0][:])
                                nc.sync.dma_start(dbg_d[4 + j], dt2[:])
                            pv_pend.append((gs, pt))
                        pgs, ppt = pv_pend.pop()
                        for p_ in range(2):
                            nc.tensor.matmul(
                                opair[p_][0:65, :],
                                vaug[pgs // 128][:], ppt[p_][:],
                                start=(pgs == b * T + s0lo), stop=True)
                        if DBG and b == 0 and i == 0:
                            for p_ in range(2):
                                dt3 = wk.tile([65, 2 * TQ], dt, tag="dbg3",
                                              name="dbg3", bufs=2)
                                nc.vector.tensor_copy(dt3[:], opair[p_][:])
                                nc.sync.dma_start(
                                    dbg_d[6 + p_, 0:65, :], dt3[:])
                        # softmax divide: rows 64 hold the denominators
                        for p in range(2):
                            zd = wk.tile([1, 2 * TQ], dt, tag="zd", bufs=1)
                            nc.vector.tensor_copy(zd[:], opair[p][64:65, :])
                            zf = wk.tile([1, 2 * TQ], dt, tag="zf", bufs=1)
                            nc.vector.reciprocal_approx_fast(zf[:], zd[:])
                            zr = wk.tile([1, 2 * TQ], dtr, tag="zr", bufs=1)
                            nc.vector.tensor_copy(zr[:], zf[:])
                            zbc = pz.tile([64, 2 * TQ], dt, tag="zb")
                            nc.tensor.matmul(zbc[:], ones_row[:], zr[:],
                                             start=True, stop=True)
                            zbs = wk.tile([64, 2 * TQ], dt, tag="zbs", bufs=1)
                            nc.any.tensor_copy(zbs[:], zbc[:])
                            for h2 in range(2):
                                nc.any.tensor_mul(
                                    o2[p][h2 * 64:(h2 + 1) * 64, g0:g0 + TQ],
                                    opair[p][0:64, h2 * TQ:(h2 + 1) * TQ],
                                    zbs[:, h2 * TQ:(h2 + 1) * TQ])
                            if DBG and b == 0 and i == 0:
                                dt4 = wk.tile([64, 2 * TQ], dt, tag="dbg4",
                                              name="dbg4", bufs=2)
                                nc.vector.tensor_copy(dt4[:], zbs[:])
                                nc.sync.dma_start(
                                    dbg_d[8 + p, 0:64, :], dt4[:])

            # ================= phase 5: wo projection =================
            with tc.tile_pool(name="ps_wo", bufs=8, space="PSUM") as pw:
                for ot in range(NKT):
                    osl = slice(ot * 128, (ot + 1) * 128)
                    for half in range(2):
                        wps = [(pw.tile([128, TC], dt, tag="wo", name="wp"),
                                slice((half * 4 + t_) * TC,
                                      (half * 4 + t_ + 1) * TC))
                               for t_ in range(4)]
                        for p in range(2):
                            for wp, csl in wps:
                                nc.tensor.matmul(
                                    wp[:], wo_sb[p][:, osl], o2[p][:, csl],
                                    start=(p == 0), stop=(p == 1))
                        for wp, csl in wps:
                            ost = op_.tile([128, TC], bf, tag="os")
                            nc.any.tensor_copy(ost[:], wp[:])
                            nc.sync.dma_start(out_d[osl, csl], ost[:])
    nc.finalize()
    return nc


def _host_inputs(x, wq, wk, wv, wo, q_norm_w, k_norm_w):
    import ml_dtypes
    f32 = np.float32
    bf16 = ml_dtypes.bfloat16
    x = np.asarray(x, f32)
    xTb = np.ascontiguousarray(x.reshape(T4, DIM).T).astype(bf16)
    r = 2.0 ** (-8.0 / NH)
    slopes = np.asarray([r ** i for i in range(NH)], f32)
    sc_col = np.concatenate([np.full((64, 64), 1.0 / 32.0, f32),
                             np.full((64, 64), 1.0 / 512.0, f32)])
    bi_col = np.concatenate([np.full((64, 64), 64.0 * EPS, f32),
                             np.full((64, 64), EPS, f32)])
    # masks [4, 128, 512]: local key row sl, local query col ql (x2 heads).
    # 0: M_C oldest block (j=0, t0>=W):   mask sl <  ql
    # 1: M_A diag block  (j=nblk-2):      mask sl >  ql
    # 2: M_B diag block  (j=nblk-1):      mask sl+128 > ql
    # 3: M_D 2nd-oldest  (j=1, t0>=W):    mask sl+128 < ql
    sl = np.arange(128)[:, None]
    ql = np.arange(TQ)[None, :]
    NEG = f32(-1e30)
    m0 = np.where(sl < ql, NEG, 0.0).astype(f32)
    m1 = np.where(sl > ql, NEG, 0.0).astype(f32)
    m2 = np.where(sl + 128 > ql, NEG, 0.0).astype(f32)
    m3 = np.where(sl + 128 < ql, NEG, 0.0).astype(f32)
    masks = np.stack([np.concatenate([m, m], 1) for m in (m0, m1, m2, m3)])
    # positions centered at -1024 so |slope*s| stays <= 1024*slope: the
    # fp22 rounding of the s-row then matches the bias-table error envelope.
    # The -slope*t row is constant per query column and cancels in softmax.
    tpos = (np.arange(T4) % T).astype(f32) - f32(WINDOW)
    in_maps = []
    for c in range(8):
        wTc = np.concatenate([
            np.asarray(wq, f32)[c * QD:(c + 1) * QD],
            np.asarray(wk, f32)[c * HD:(c + 1) * HD],
            np.asarray(wv, f32)[c * HD:(c + 1) * HD]], 0).T
        wTc = np.ascontiguousarray(wTc).astype(bf16)
        woTc = np.ascontiguousarray(
            np.asarray(wo, f32)[:, c * QD:(c + 1) * QD].T).astype(bf16)
        aug = np.zeros((10, T4), f32)
        aug[0] = tpos
        aug[1] = 1.0
        for h in range(QH):
            aug[2 + h] = slopes[4 * c + h]
            aug[6 + h] = -slopes[4 * c + h] * tpos
        in_maps.append({
            "xT": xTb,
            "wT": wTc,
            "woT": woTc,
            "qnw": np.asarray(q_norm_w, f32)[c * QD:(c + 1) * QD]
            .reshape(1, QD),
            "knw": np.asarray(k_norm_w, f32)[c * HD:(c + 1) * HD]
            .reshape(1, HD),
            "aug": aug,
            "masks": masks,
            "ones_col": np.ones((128, 1), f32),
            "ones_row": np.ones((1, 64), f32),
            "ident": np.eye(64, dtype=f32),
            "sc_col": sc_col,
            "bi_col": bi_col,
        })
    return in_maps


def kernel(x, wq, wk, wv, wo, q_norm_w, k_norm_w):
    from concourse.bass_utils import run_bass_kernel_spmd
    if "nc" not in _CACHE:
        _CACHE["nc"] = _build_bass()
    nc = _CACHE["nc"]
    in_maps = _host_inputs(x, wq, wk, wv, wo, q_norm_w, k_norm_w)
    res = run_bass_kernel_spmd(nc, in_maps, core_ids=list(range(8)))
    out = np.zeros((DIM, T4), np.float64)
    for c in range(8):
        out += res.results[c]["out"].astype(np.float64)
    return np.ascontiguousarray(out.T).reshape(B, T, DIM).astype(np.float32)
